# revision 69
# baseline (speedup 1.0000x reference)
"""Trainium2 Bass kernel for nn_DiverseRegDCConv2d.

Per-sample dynamic 3x3 conv: filters are generated per sample from an
8-column weight bank (wgen[b] = se[b] @ bank.T), then applied as a
standard 256->256 conv on 28x28 with padding 1.

Sharding (8 cores): 4 batch-groups x 2 out-channel halves. Each core
handles 8 samples x 128 out channels; the weight bank half it needs is
replicated across the 4 batch-groups. No cross-device communication.

Compute strategy (fp8 DoubleRow, 3-pass residual):
  The conv contraction (256 input channels = 2x128) runs as fp8
  DoubleRow matmuls, which process both 128-channel tiles in a single
  instruction at 2x the fp16 rate (4x overall).  Plain e4m3 is not
  accurate enough (rel err 3.7e-2 vs the 2e-2 gate), so operands are
  split hi/lo and three DoubleRow passes accumulate into one PSUM
  group:
      P1: Whi . xhi     (e4m3 x e4m3)  main term
      P2: Whi . xlo     (e4m3 x e4m3)  x-quantization correction
      P3: Wlo . xhi     (e5m2 x e4m3)  W-quantization correction
  Wlo must be e5m2: the residual magnitudes (~1e-3) underflow e4m3's
  denormal range.  Measured end-to-end rel err: ~1.6e-3.

  x windows stream as FLAT slices of a row-contiguous [128, cc, 31, 30]
  image (padded, plus a zero guard row), offset by ky*30+kx per kernel
  position; the wrap-around junk lands in output columns 28/29 which
  the evacuation discards (the final row's junk pair is not even
  computed: N = nr*30-2).  Flatness is required by the DoubleRow ifmap
  layout ([K, 2, N] with N flat).

  Filters are generated on-device (bank tiles x f16 block-diagonal se
  operand -> PSUM fp32), then split into the e4m3 hi slab (activation
  copy) and e5m2 lo slab (tensor_tensor subtract) so the split keeps
  pace with the PE during the generation phase.

Schedule (TimelineSim-tuned; knobs below were swept):
  Phase 1 interleaves the 18 bank-tile loads, x tiles 0-4, wgen rounds,
  and five 17-row conv bands (one per wave sample) with per-group lags
  matched to x arrival; phase 2 runs the remaining 11 bands plus a
  6+5-row split of the final band so the last evac+store chain is
  short.  The startup (~3.5us) and tail (~3.3us) sit at the DMA latency
  floor (HWDGE issue 625ns + DGE dispatch 650ns + completion semaphore
  900ns); the PE itself is busy 45.3us of the 58.9us total, which is
  the algorithmic floor for the 3-pass DoubleRow scheme.
"""

import sys

for _p in ("/opt/trn_rl_repo", "/root/.axon_site/_ro/trn_rl_repo"):
    if _p not in sys.path:
        sys.path.append(_p)

import numpy as np
import ml_dtypes

import concourse.bass as bass
import concourse.mybir as mybir
from concourse import bacc
from concourse.bass_utils import run_bass_kernel_spmd
from concourse.tile import TileContext

B, C, O, KS, H, W, NUM = 32, 256, 256, 3, 28, 28, 8
P = 128
NCORES = 8
BG, OHALF = 4, 2          # batch-groups x out-channel halves
S = B // BG               # samples per core = 8
OC = O // OHALF           # out channels per core = 128
CC = C // P               # input-channel chunks = 2
G = 16                    # (k,o)-blocks per wgen matmul (with NUM=8 fills K=128)
NP = KS * KS * OC         # (k, o_local) pairs per c-chunk = 1152
NM = NP // G              # wgen matmuls per c-chunk = 72
WR, WC = H + 3, W + 2     # stored x rows (pad + zero guard row) x cols = 31x30
HH = H // 2               # output rows per PSUM group = 14
NW = HH * WC              # flat moving-stream width = 420
F32 = mybir.dt.float32
F16 = mybir.dt.float16
E3 = mybir.dt.float8e3
E4 = mybir.dt.float8e4
E5 = mybir.dt.float8e5
IDENT = mybir.ActivationFunctionType.Identity
SUB = mybir.AluOpType.subtract

# -- schedule tuning knobs ------------------------------------------------
TAILBANDS = ((17, 6), (23, 5))
WAVE1 = 5          # conv groups interleaved with the wgen phase (PSUM tags)
WGTAGS = 3         # wgen PSUM rotation depth (WAVE1 + WGTAGS <= 8)
LAGS = (1, 1, 1, 2, 4)    # per-wave-group P1/P2 lag behind the wgen round
LAG3OFF = 1        # P3 lag relative to P1/P2
EVAC0 = 0          # (unused, kept for config compat)
EVACPAT = "ad" * 9        # per-evac engine pattern (a: Act, d: DVE)
TAILEVAC = "dve"   # engine for the final band evacs
TAGRESET = 0       # reset conv psum tag rotation at phase 2 start
P2ORDER = 0        # 0: ..s6p0,s6p1,s7p0 ; 1: ..s6p0,s7p0,s6p1
OUTBUFS = 4        # outp pool buffers
XLAST = 4          # last x tile loaded inside the bank-stream loop
WAVEKIND = 0       # phase-1 wave composition (see below)
BIASK = 6          # bank-stream round at which the bias DMA is issued (>8: last)
WGORDER = 0        # position of wgen within each round (0: first, 1: mid, 2: last)
KMAJOR = 0         # 1: one bank DMA per round covering both cc chunks

_NC = None


def _build_nc():
    nc = bacc.Bacc()
    x_d = nc.declare_dram_parameter("x", [S, 2, CC, P, WR * WC], E4, isOutput=False)
    # bank tiles, e3m4 scaled x32 (un-scaled at the PSUM evacuation)
    wp_d = nc.declare_dram_parameter("wp", [P, CC * NM, P], E3, isOutput=False)
    se_d = nc.declare_dram_parameter("sebd", [P, P], F16, isOutput=False)
    b_d = nc.declare_dram_parameter("bias", [P, 1], F32, isOutput=False)
    out_d = nc.declare_dram_parameter("out", [S, P, H * W], F16, isOutput=True)

    with TileContext(nc) as tc:
        with (
            tc.tile_pool(name="constp", bufs=1) as constp,
            tc.tile_pool(name="wstream", bufs=18) as wstream,
            tc.tile_pool(name="slabp", bufs=1) as slabp,
            tc.tile_pool(name="xpool", bufs=1) as xpool,
            tc.tile_pool(name="outp", bufs=OUTBUFS) as outp,
            tc.tile_pool(name="psp", bufs=1, space="PSUM") as psp,
        ):
            # se rides the Pool/SWDGE queue so its issue overhead overlaps
            # the HWDGE chain of the first weight-block load
            se_sb = constp.tile([P, P], F16)
            nc.gpsimd.dma_start(out=se_sb, in_=se_d[:, :])
            bias_sb = constp.tile([P, 1], F32)  # DMA deferred past startup

            # filter slabs: [c_part, cc, k, s, o]; conv lhsT slices are
            # wg*[:, :, k, s, :] = [128, 2, 128], the DoubleRow pair shape.
            wg_hi = slabp.tile([P, CC, KS * KS, S, P], E4)
            wg_lo = slabp.tile([P, CC, KS * KS, S, P], E5)

            # x tiles: [c_part, hi/lo, cc, 930] flat row-contiguous images
            xt = [
                xpool.tile([P, 2, CC, WR * WC], E4, name=f"xt_{s}", tag=f"xt_{s}")
                for s in range(S)
            ]
            xdone = set()

            def emit_xload(s):
                if s in xdone:
                    return
                xdone.add(s)
                nc.sync.dma_start(
                    out=xt[s], in_=x_d[s].rearrange("hl cc p w -> p hl cc w"),
                )

            def emit_wload(cc, k, split=False):
                t0 = cc * NM + k * 8
                wtb = wstream.tile([P, 8, P], E3, name=f"wtb_{cc}_{k}", tag="wtb")
                if split:
                    # halves land separately so the first wgen group starts
                    # as soon as tiles 0-3 arrive
                    nc.sync.dma_start(out=wtb[:, 0:4, :], in_=wp_d[:, t0:t0 + 4, :])
                    nc.sync.dma_start(out=wtb[:, 4:8, :], in_=wp_d[:, t0 + 4:t0 + 8, :])
                else:
                    nc.sync.dma_start(out=wtb, in_=wp_d[:, t0:t0 + 8, :])
                return wtb

            # wgen psum rotates over 3 of the shared pool's 8 bank tags;
            # phase-2 conv groups inherit those tags once wgen drains
            _splitn = [0]

            def emit_wgen(cc, k, wtb):
                # produce wg_hi/wg_lo[:, cc, k, :, :] (8 o_hi blocks = 2 groups)
                for j in range(2):
                    n = _splitn[0]
                    _splitn[0] += 1
                    ps = psp.tile([P, 4 * P], F32, name=f"wgps_{n}",
                                  tag=f"wg{n % WGTAGS}")
                    for i in range(4):
                        nc.tensor.matmul(
                            ps[:, i * P:(i + 1) * P], wtb[:, j * 4 + i, :],
                            se_sb, start=True, stop=True,
                        )
                    # psum free layout: (o_hi, s, g); slab wants (s, o_hi, g)
                    oh0 = j * 4
                    src = ps.rearrange("p (oh s g) -> p oh s g", oh=4, s=S, g=G)
                    dhi = wg_hi[:, cc, k, :, oh0 * G:(oh0 + 4) * G].rearrange(
                        "p s (oh g) -> p oh s g", g=G)
                    dlo = wg_lo[:, cc, k, :, oh0 * G:(oh0 + 4) * G].rearrange(
                        "p s (oh g) -> p oh s g", g=G)
                    nc.scalar.activation(dhi, src, IDENT)
                    nc.vector.tensor_tensor(dlo, src, dhi, SUB)

            # conv psum tag order: 5 dedicated tags during phase 1, then the
            # freed wgen tags join the rotation (8 banks deep in phase 2).
            # Groups are asymmetric row-bands: part 0 = rows 0-16 (N=510,
            # just under the 2KB PSUM bank), part 1 = rows 17-27 (N=330).
            # The tall part-0 bands maximize conv work that fits in the 5
            # phase-1 banks while the split chain paces the window.

            R0 = {0: 0, 1: 17}
            NR = {0: 17, 1: 11}
            _cv_tags = [f"cv{i}" for i in range(WAVE1)] + [f"wg{i}" for i in range(WGTAGS)]
            _tag = [0]

            def conv_psum(part):
                t = psp.tile([P, NR[part] * WC], F32, name=f"cps_{_tag[0]}",
                              tag=_cv_tags[_tag[0] % len(_cv_tags)])
                _tag[0] += 1
                return t

            def emit_conv_mm(k, s, part, pst, pas, first, last):
                # pas 0: Whi.xhi  1: Whi.xlo  2: Wlo.xhi
                # the last row's 2 junk columns are never read -> skip them
                ky, kx = k // KS, k % KS
                off = (R0[part] + ky) * WC + kx
                nn = NR[part] * WC - 2
                hl = 1 if pas == 1 else 0
                wg = wg_lo if pas == 2 else wg_hi
                nc.tensor.matmul(
                    pst[:, 0:nn], wg[:, :, k, s, :],
                    xt[s][:, hl, :, off:off + nn],
                    start=first, stop=last,
                    perf_mode=mybir.MatmulPerfMode.DoubleRow,
                    skip_group_check=True,
                )

            _evacn = [0]
            _ots = {}

            def emit_group_evac(s, part, pst):
                if s not in _ots:
                    _ots[s] = outp.tile([P, H * W], F16, name=f"ot_{s}",
                                        tag=f"ot_{s % OUTBUFS}")
                ot = _ots[s]
                nr, r0 = NR[part], R0[part]
                src = pst.rearrange("p (r c) -> p r c", r=nr)[:, :, 0:W]
                dst = ot[:, r0 * W:(r0 + nr) * W].rearrange("p (r c) -> p r c", r=nr)
                if EVACPAT[_evacn[0] % len(EVACPAT)] == "a":
                    nc.scalar.activation(dst, src, IDENT,
                                         bias=bias_sb[:, 0:1], scale=1.0 / 32)
                else:
                    nc.vector.tensor_scalar(
                        out=dst, in0=src, scalar1=1.0 / 32,
                        scalar2=bias_sb[:, 0:1], op0=mybir.AluOpType.mult,
                        op1=mybir.AluOpType.add)
                _evacn[0] += 1
                if s == S - 1:
                    nc.sync.dma_start(
                        out=out_d[s, :, r0 * W:(r0 + nr) * W], in_=ot[:, r0 * W:(r0 + nr) * W])
                elif part == 1:
                    nc.sync.dma_start(out=out_d[s], in_=ot)
                return ot

            # ---- phase 1: wgen + a conv wave, interleaved.  WAVEKIND 0 =
            # part-0 bands of the first WAVE1 samples; 1 = full sample
            # pairs (fewer x tiles needed early) ----
            if WAVEKIND == 0:
                wave1 = [(s, 0) for s in range(WAVE1)]
            elif WAVEKIND == 1:
                wave1 = [(0, 0), (0, 1), (1, 0), (1, 1), (2, 0)][:WAVE1]
            else:
                wave1 = [(0, 0), (0, 1), (1, 0), (2, 0), (3, 0)][:WAVE1]
            prog = {}
            done_k = {}

            def open_group(g):
                prog[g] = conv_psum(g[1])
                done_k[g] = 0

            def conv_step(g, k, passes):
                s, part = g
                pst = prog[g]
                for pas in passes:
                    idx = done_k[g]
                    emit_conv_mm(k, s, part, pst, pas,
                                 first=(idx == 0), last=(idx == 27 - 1))
                    done_k[g] += 1

            for g in wave1:
                open_group(g)

            # DMA priority order: weight pairs with x tiles interleaved;
            # x tiles past XLAST wait until the bank stream is done so they
            # never starve the wgen rounds.  KMAJOR batches both cc chunks
            # of a round into one DMA (halves the 625ns HWDGE issue cost).
            wtbs = {}
            if KMAJOR:
                wtb0 = wstream.tile([P, 16, P], E3, name="wtbk_0", tag="wtbk")
                nc.sync.dma_start(out=wtb0[:, 0:4, :], in_=wp_d[:, 0:4, :])
                nc.sync.dma_start(out=wtb0[:, 4:16, :], in_=wp_d[:, 4:16, :])
                wtbs[(0, 0)], wtbs[(1, 0)] = wtb0[:, 0:8], wtb0[:, 8:16]
            else:
                wtbs[(0, 0)] = emit_wload(0, 0, split=True)
                wtbs[(1, 0)] = emit_wload(1, 0)
            emit_xload(0)
            for k in range(1, KS * KS):
                if KMAJOR:
                    wtbk = wstream.tile([P, 16, P], E3, name=f"wtbk_{k}",
                                        tag="wtbk")
                    nc.sync.dma_start(out=wtbk, in_=wp_d[:, k * 16:k * 16 + 16, :])
                    wtbs[(0, k)], wtbs[(1, k)] = wtbk[:, 0:8], wtbk[:, 8:16]
                else:
                    wtbs[(0, k)] = emit_wload(0, k)
                    wtbs[(1, k)] = emit_wload(1, k)
                if k <= XLAST:
                    emit_xload(k)
                if k == BIASK:
                    nc.sync.dma_start(out=bias_sb, in_=b_d[:, :])
            for s in range(XLAST + 1, S):
                emit_xload(s)
            if BIASK > 8:
                nc.sync.dma_start(out=bias_sb, in_=b_d[:, :])

            # staggered lags: later bands wait for their x tiles
            lag12 = {g: LAGS[i] for i, g in enumerate(wave1)}
            lag3 = {g: lag12[g] + LAG3OFF for g in wave1}
            for k in range(KS * KS):
                if WGORDER == 0:
                    emit_wgen(0, k, wtbs[(0, k)])
                    emit_wgen(1, k, wtbs[(1, k)])
                for g in wave1:
                    if k >= lag12[g]:
                        conv_step(g, k - lag12[g], (0, 1))
                if WGORDER == 1:
                    emit_wgen(0, k, wtbs[(0, k)])
                    emit_wgen(1, k, wtbs[(1, k)])
                for g in wave1:
                    if k >= lag3[g]:
                        conv_step(g, k - lag3[g], (2,))
                if WGORDER == 2:
                    emit_wgen(0, k, wtbs[(0, k)])
                    emit_wgen(1, k, wtbs[(1, k)])
            # drain wave-1 per group so PSUM tags free progressively
            for g in wave1:
                for kk in range(KS * KS - lag12[g], KS * KS):
                    conv_step(g, kk, (0, 1))
                for kk in range(KS * KS - lag3[g], KS * KS):
                    conv_step(g, kk, (2,))
                emit_group_evac(*g, prog[g])

            # ---- phase 2: part-1 bands of wave samples, then full pairs ----
            if TAGRESET:
                _tag[0] = 0
            rest = [g for s in range(S) for g in ((s, 1), (s, 0))
                    if g not in wave1 and g != (S - 1, 1)]
            if WAVEKIND == 0:
                rest.sort(key=lambda g: (g[0], g[1]) if g[0] >= WAVE1 else (-1, -g[1]))
            else:
                rest.sort()
            rest.append((S - 1, 1))
            if P2ORDER == 1 and len(rest) >= 3:
                rest[-1], rest[-2] = rest[-2], rest[-1]
            for g in rest[:-1]:
                open_group(g)
                for k in range(KS * KS):
                    conv_step(g, k, (0, 1, 2))
                emit_group_evac(*g, prog[g])

            # final band runs as a 9-row band plus a tiny 2-row band so the
            # last matmul->evac chain is short; ONE store covers both (two
            # stores would serialize 625ns HWDGE slots onto the tail)
            s = S - 1
            ot = _ots[s]
            for r0, nr in TAILBANDS:
                pst = psp.tile([P, nr * WC], F32, name=f"cps_t{r0}",
                               tag=_cv_tags[(_tag[0] + (r0 != 17)) % len(_cv_tags)])
                nn = nr * WC - 2
                for k in range(KS * KS):
                    ky, kx = k // KS, k % KS
                    off = (r0 + ky) * WC + kx
                    for pas in range(3):
                        hl = 1 if pas == 1 else 0
                        wg = wg_lo if pas == 2 else wg_hi
                        nc.tensor.matmul(
                            pst[:, 0:nn], wg[:, :, k, s, :],
                            xt[s][:, hl, :, off:off + nn],
                            start=(k == 0 and pas == 0),
                            stop=(k == KS * KS - 1 and pas == 2),
                            perf_mode=mybir.MatmulPerfMode.DoubleRow,
                            skip_group_check=True,
                        )
                src = pst.rearrange("p (r c) -> p r c", r=nr)[:, :, 0:W]
                dst = ot[:, r0 * W:(r0 + nr) * W].rearrange("p (r c) -> p r c", r=nr)
                if TAILEVAC == "act":
                    nc.scalar.activation(dst, src, IDENT,
                                         bias=bias_sb[:, 0:1], scale=1.0 / 32)
                else:
                    nc.vector.tensor_scalar(
                        out=dst, in0=src, scalar1=1.0 / 32,
                        scalar2=bias_sb[:, 0:1], op0=mybir.AluOpType.mult,
                        op1=mybir.AluOpType.add)
                nc.sync.dma_start(
                    out=out_d[s, :, r0 * W:(r0 + nr) * W],
                    in_=ot[:, r0 * W:(r0 + nr) * W])

    nc.compile()
    return nc


def _get_nc():
    global _NC
    if _NC is None:
        _NC = _build_nc()
    return _NC


def _prep_core_inputs(inputs, inputs_se, weight, bias, bg, oh):
    # weight rows: r = o*(C*9) + c*9 + (ky*3+kx)  -> [O, C, 3, 3, NUM]
    wr = weight.reshape(O, C, KS, KS, NUM)
    wo = wr[oh * OC:(oh + 1) * OC]            # [128, 256, 3, 3, 8]
    p_arr = np.arange(NP)
    k_arr = p_arr // OC                       # k index per (m,g) pair
    o_arr = p_arr % OC
    t = wo[o_arr, :, k_arr // KS, k_arr % KS, :]     # [1152, 256, 8]
    if KMAJOR:
        wp = (
            t.reshape(KS * KS, 8, G, CC, P, NUM)  # kk, oh, g, cc, c, n
            .transpose(0, 3, 1, 5, 2, 4)          # kk, cc, oh, n, g, c
            .reshape(CC * NM, P, P)
            .transpose(1, 0, 2)                   # p-major for contiguous DMA
        )
    else:
        wp = (
            t.reshape(NM, G, CC, P, NUM)
            .transpose(2, 0, 4, 1, 3)             # cc, m, n, g, c
            .reshape(CC * NM, P, P)
            .transpose(1, 0, 2)                   # p-major for contiguous DMA
        )
    wp = np.ascontiguousarray((wp * 32.0).astype(ml_dtypes.float8_e3m4))

    se_core = inputs_se[bg * S:(bg + 1) * S]  # [8, 8] (s, n)
    sebd = np.zeros((NUM, G, S, G), dtype=np.float32)
    for g in range(G):
        sebd[:, g, :, g] = se_core.T
    sebd = sebd.reshape(P, P).astype(np.float16)

    # padded x + zero guard row, flat rows; hi/lo e4m3 split
    xp = np.pad(
        inputs[bg * S:(bg + 1) * S], ((0, 0), (0, 0), (1, 2), (1, 1))
    ).reshape(S, CC, P, WR * WC)
    xhi = xp.astype(ml_dtypes.float8_e4m3)
    xlo = (xp - xhi.astype(np.float32)).astype(ml_dtypes.float8_e4m3)
    x_arr = np.stack([xhi, xlo], axis=1)      # [S, 2, CC, P, 930]

    return {
        "x": np.ascontiguousarray(x_arr),
        "wp": wp,
        "sebd": sebd,
        "bias": np.ascontiguousarray(
            bias[oh * OC:(oh + 1) * OC].reshape(OC, 1), dtype=np.float32
        ),
    }


def kernel(inputs, inputs_se, weight, bias):
    inputs = np.asarray(inputs, dtype=np.float32)
    inputs_se = np.asarray(inputs_se, dtype=np.float32)
    weight = np.asarray(weight, dtype=np.float32)
    bias = np.asarray(bias, dtype=np.float32)

    nc = _get_nc()
    in_maps = []
    for core in range(NCORES):
        bg, oh = core // OHALF, core % OHALF
        in_maps.append(_prep_core_inputs(inputs, inputs_se, weight, bias, bg, oh))

    res = run_bass_kernel_spmd(nc, in_maps, list(range(NCORES))).results

    out = np.empty((B, O, H, W), dtype=np.float32)
    for core in range(NCORES):
        bg, oh = core // OHALF, core % OHALF
        out[bg * S:(bg + 1) * S, oh * OC:(oh + 1) * OC] = (
            res[core]["out"].astype(np.float32).reshape(S, OC, H, W)
        )
    return out

